# revision 1
# baseline (speedup 1.0000x reference)
"""Trainium2 Bass kernel for CNN+GCN+MLP (nn_CNNGCN_18236431139458).

Strategy (8 NeuronCores, one chip):
  - Conv + both GCN layers: data-parallel over batch (4 samples/core).
    The scatter-aggregate is a dense matmul against the normalized
    adjacency A^T (built host-side from edge_index). A^T streams from
    HBM once per layer (samples innermost), overlapped with PE.
    Layer-1 aggregation runs in fp8 (DoubleRow, 2x PE rate) — its
    quantization noise is coherently averaged away by the layer-2
    aggregation (A >= 0, post-relu h >= 0). Layer-2 stays bf16 since
    nothing downstream washes its noise out.
  - MLP: W1 (262144 x 100) is sharded over rows (nodes) across cores.
    An on-device AllToAll reshards the GCN output from batch-sharded to
    node-sharded; each core computes a partial [32, 100] with its W1
    shard; a ReduceScatter sums partials and hands each core its own 4
    samples for the tiny MLP tail.
  - All other matmuls bf16 with f32 PSUM accumulation; tail in f32.

Layouts (per core):
  xT   [128 ic, 4 s, 2050]   feature-major input slices
  h*T  [128 f, 4 s, 2048 n]  feature-major activations
  hw   [128 n, 16 nch, 4*128] node-major GCN linear outputs
  at   [16 sc, 128 p, 2048 dst] = A^T[sc*128+p, dst]  (streamed rhs)
  w1s  [128 f, 256 n, 100]   W1 row-shard (this core's 256 nodes)
"""

import numpy as np
import ml_dtypes

import concourse.bass as bass
import concourse.mybir as mybir
import concourse.tile as tile
from concourse.tile import add_dep_helper
from concourse import bacc
from concourse.bass_utils import run_bass_kernel_spmd

BF16 = mybir.dt.bfloat16
FP8 = mybir.dt.float8e4
F32 = mybir.dt.float32
NP_BF16 = ml_dtypes.bfloat16
NP_FP8 = mybir.dt.np(FP8)

B, H, E = 32, 2050, 128
N = 2048
C = 128
G1 = G2 = 128
MLPD = 100
KS = 3
NE = 32768
NCORES = 8
BL = B // NCORES          # 4 samples per core
NSH = N // NCORES         # 256 nodes per core (W1 row shard)
RG = [list(range(NCORES))]

Relu = mybir.ActivationFunctionType.Relu
DoubleRow = mybir.MatmulPerfMode.DoubleRow


def _emit_front(nc, tc, pools, tensors):
    """conv + GCN1 + GCN2 -> h2T [128 f, 4 s, 2048 n] bf16."""
    acts, psum, apool = pools["acts"], pools["psum"], pools["apool"]
    xT_sb = tensors["xT_sb"]
    wc_sb = tensors["wc_sb"]
    cb_sb = tensors["cb_sb"]

    # ---- conv: h0T[oc, n] = relu(sum_k WcT_k.T @ xT[:, n+k] + cb) ----
    h0T = acts.tile([128, BL, N], BF16, tag="hT", bufs=2, name="h0T")
    for nt in range(4):
        for s in range(BL):
            ps = psum.tile([128, 512], F32, tag="ps", name="ps_conv")
            for k in range(KS):
                nc.tensor.matmul(
                    ps[:],
                    lhsT=wc_sb[:, k, :],
                    rhs=xT_sb[:, s, nt * 512 + k : nt * 512 + k + 512],
                    start=(k == 0),
                    stop=(k == KS - 1),
                )
            act = nc.scalar.activation(h0T[:, s, nt * 512 : (nt + 1) * 512], ps[:], Relu, bias=cb_sb[:])
            if s == 0 and nt == 0:
                tensors["anchor_conv0"] = act

    # ---- GCN layer 1: linear + fp8 DoubleRow aggregation (A8 streamed) ----
    hw1 = acts.tile([128, 16, BL * 128], FP8, tag="hw8", bufs=1, name="hw1")
    for nch in range(16):
        ps = psum.tile([128, 512], F32, tag="ps", name="ps_lin1")
        for s in range(BL):
            nc.tensor.matmul(
                ps[:, s * 128 : (s + 1) * 128],
                lhsT=h0T[:, s, nch * 128 : (nch + 1) * 128],
                rhs=tensors["gw1_sb"][:],
                start=True,
                stop=True,
            )
        nc.vector.tensor_copy(hw1[:, nch, :], ps[:])

    h1T = acts.tile([128, BL, N], BF16, tag="hT", bufs=2, name="h1T")
    at8 = tensors["at8"]
    for dt in range(4):
        pss = [psum.tile([128, 512], F32, tag="ps", name=f"ps_agg{s}")
               for s in range(BL)]
        for sc2 in range(8):
            atile = apool.tile([128, 2, 512], FP8, tag="atile", name="atile")
            nc.sync.dma_start(
                atile[:],
                at8[2 * sc2 : 2 * sc2 + 2, :, dt * 512 : (dt + 1) * 512]
                .rearrange("c p d -> p c d"),
            )
            for s in range(BL):
                nc.tensor.matmul(
                    pss[s][:],
                    lhsT=hw1[:, 2 * sc2 : 2 * sc2 + 2, s * 128 : (s + 1) * 128],
                    rhs=atile[:],
                    start=(sc2 == 0),
                    stop=(sc2 == 7),
                    perf_mode=DoubleRow,
                )
        for s in range(BL):
            act = nc.scalar.activation(h1T[:, s, dt * 512 : (dt + 1) * 512],
                                       pss[s][:], Relu, bias=tensors["gb1_sb"][:])
            tensors["anchor_agg1_end"] = act

    # ---- GCN layer 2: linear + bf16 aggregation (A resident in SBUF),
    #      sample-outer so each sample's h2T finishes early for the A2A ----
    hw2 = acts.tile([128, 16, BL * 128], BF16, tag="hw2", bufs=1, name="hw2")
    for nch in range(16):
        ps = psum.tile([128, 512], F32, tag="ps", name="ps_lin2")
        for s in range(BL):
            nc.tensor.matmul(
                ps[:, s * 128 : (s + 1) * 128],
                lhsT=h1T[:, s, nch * 128 : (nch + 1) * 128],
                rhs=tensors["gw2_sb"][:],
                start=True,
                stop=True,
            )
        nc.vector.tensor_copy(hw2[:, nch, :], ps[:])

    A_sb = tensors["A_sb"]
    # per-sample tiles so each sample's A2A staging only depends on its own
    # aggregation output
    h2Ts = []
    for s in range(BL):
        h2T_s = acts.tile([128, N], BF16, tag=f"h2T{s}", name=f"h2T{s}")
        pss = [psum.tile([128, 512], F32, tag="ps", name=f"ps_agg2_{dt}")
               for dt in range(4)]
        for sc in range(16):
            for dt in range(4):
                nc.tensor.matmul(
                    pss[dt][:],
                    lhsT=hw2[:, sc, s * 128 : (s + 1) * 128],
                    rhs=A_sb[:, sc, dt * 512 : (dt + 1) * 512],
                    start=(sc == 0),
                    stop=(sc == 15),
                )
        for dt in range(4):
            nc.scalar.activation(h2T_s[:, dt * 512 : (dt + 1) * 512],
                                 pss[dt][:], Relu, bias=tensors["gb2_sb"][:])
        h2Ts.append(h2T_s)

    return h2Ts


def _emit_tail(nc, tc, pools, tensors, h2Ts, out_ap, collectives=True):
    """AllToAll reshard + sharded MLP + ReduceScatter + local MLP tail."""
    acts, psum, psum2, dram, small, wpool = (
        pools["acts"], pools["psum"], pools["psum2"], pools["dram"],
        pools["small"], pools["wpool"],
    )
    w1s = tensors["w1s"]

    # One AllToAll per local sample, issued as soon as that sample's h2T
    # rows are done — staging + wire hide under the next sample's agg2.
    h2a = acts.tile([128, B, NSH], BF16, tag="h2a", name="h2a")
    for s in range(BL):
        a2a_in = dram.tile([NCORES, 128, NSH], BF16, tag=f"a2a_in{s}",
                           name=f"a2a_in{s}")
        a2a_out = dram.tile([NCORES, 128, NSH], BF16, tag=f"a2a_out{s}",
                            name=f"a2a_out{s}")
        for j in range(NCORES):
            nc.sync.dma_start(a2a_in[j], h2Ts[s][:, j * NSH : (j + 1) * NSH])
        if collectives:
            nc.gpsimd.collective_compute(
                "AllToAll", mybir.AluOpType.bypass, replica_groups=RG,
                ins=[a2a_in.opt()], outs=[a2a_out.opt()],
            )
            for i in range(NCORES):
                nc.sync.dma_start(h2a[:, i * BL + s, :], a2a_out[i])
        else:
            # timing stand-in: skip the wire, read staged data directly
            # (the real collective's cost is measured separately)
            for i in range(NCORES):
                nc.sync.dma_start(h2a[:, i * BL + s, :], a2a_in[i])

    # PE warmers: keep the tensor engine busy across the A2A wait so the
    # MLP matmuls run at full (ramped) clock. Results are garbage but kept
    # live via a DMA side effect.
    n_warm = tensors.get("n_warm", 20)
    if n_warm > 0:
        warm_ps = psum.tile([128, 512], F32, tag="ps", name="warm_ps")
        for w in range(n_warm):
            nc.tensor.matmul(
                warm_ps[:],
                lhsT=tensors["gw2_sb"][:],
                rhs=tensors["A_sb"][:, 0, 0:512],
                start=(w == 0),
                stop=(w == n_warm - 1),
            )
        warm_sb = small.tile([128, 1], F32, tag="warm_sb", name="warm_sb")
        nc.vector.tensor_copy(warm_sb[:], warm_ps[:, 0:1])
        warm_dr = dram.tile([128, 1], F32, tag="warm_dr", name="warm_dr")
        nc.sync.dma_start(warm_dr[:], warm_sb[:])

    # z[b, c] = sum_n  h2a[:, :, n].T @ w1s[:, n, :]   (256 k-tiles).
    # M=32 wastes 3/4 of the PE columns, so column-tile: 4 nodes run
    # concurrently in disjoint 32-col groups (tile_position), each
    # accumulating its node-subset into its own partition range.
    ps_z = psum2.tile([128, MLPD], F32, tag="psz", name="ps_z")
    WCH = 4          # stream W1 shard in 4 chunks of 64 nodes
    for ch in range(WCH):
        w1c = wpool.tile([128, NSH // WCH, MLPD], BF16, tag="w1c", bufs=2, name="w1c")
        w1c_dma = nc.sync.dma_start(w1c[:], w1s[:, ch * (NSH // WCH) : (ch + 1) * (NSH // WCH), :])
        if "anchor_agg1_end" in tensors:
            add_dep_helper(w1c_dma.ins, tensors["anchor_agg1_end"].ins,
                           reason="delay W1 prefetch past agg1's A8 stream")
        for nl in range(NSH // WCH):
            n = ch * (NSH // WCH) + nl
            j = n % 4
            nc.tensor.matmul(
                ps_z[32 * j : 32 * (j + 1), :],
                lhsT=h2a[:, :, n],
                rhs=w1c[:, nl, :],
                start=(n < 4),
                stop=(n >= NSH - 4),
                tile_position=(0, 32 * j),
            )

    z_sb = small.tile([32, MLPD], F32, tag="z_sb", name="z_sb")
    nc.vector.tensor_copy(z_sb[:], ps_z[0:32, :])
    for j in range(1, 4):
        nc.vector.tensor_add(z_sb[:], z_sb[:], ps_z[32 * j : 32 * (j + 1), :])
    rs_in = dram.tile([32, MLPD], F32, tag="rs_in", name="rs_in")
    rs_out = dram.tile([BL, MLPD], F32, tag="rs_out", name="rs_out")
    nc.sync.dma_start(rs_in[:], z_sb[:])
    zloc = small.tile([BL, MLPD], F32, tag="zloc", name="zloc")
    if collectives:
        nc.gpsimd.collective_compute(
            "ReduceScatter", mybir.AluOpType.add, replica_groups=RG,
            ins=[rs_in.opt()], outs=[rs_out.opt()],
        )
        nc.sync.dma_start(zloc[:], rs_out[:])
    else:
        nc.sync.dma_start(zloc[:], rs_in[0:BL, :])
    hm = small.tile([BL, MLPD], F32, tag="hm", name="hm")
    nc.vector.tensor_add(hm[:], zloc[:], tensors["b1r_sb"][:])
    nc.vector.tensor_scalar_max(hm[:], hm[:], 0.0)
    nc.vector.tensor_mul(hm[:], hm[:], tensors["w2r_sb"][:])
    osb = small.tile([BL, 1], F32, tag="osb", name="osb")
    nc.vector.reduce_sum(osb[:], hm[:], axis=mybir.AxisListType.X)
    nc.vector.tensor_add(osb[:], osb[:], tensors["b2r_sb"][:])
    nc.sync.dma_start(out_ap[:], osb[:])


def build_nc(front_reps=1, tail_reps=1, collectives=True, num_devices=NCORES,
             loop_all_reps=1, n_warm=0):
    """Build + compile the SPMD program. Reps>1 variants are for timing.

    loop_all_reps>1 wraps front+tail in a hardware loop with collectives
    replaced by equal-volume DMA stand-ins (collectives can't sit inside
    control flow) — used to measure whole-kernel steady-state time.
    """
    nc = bacc.Bacc("TRN2", target_bir_lowering=False, debug=False,
                   num_devices=num_devices)

    d_xT = nc.dram_tensor("xT", [BL, 128, H], BF16, kind="ExternalInput").ap()
    d_at = nc.dram_tensor("at", [16, 128, N], BF16, kind="ExternalInput").ap()
    d_at8 = nc.dram_tensor("at8", [16, 128, N], FP8, kind="ExternalInput").ap()
    d_wc = nc.dram_tensor("wc", [KS, 128, 128], BF16, kind="ExternalInput").ap()
    d_cb = nc.dram_tensor("cb", [128, 1], F32, kind="ExternalInput").ap()
    d_gw1 = nc.dram_tensor("gw1", [128, 128], BF16, kind="ExternalInput").ap()
    d_gb1 = nc.dram_tensor("gb1", [128, 1], F32, kind="ExternalInput").ap()
    d_gw2 = nc.dram_tensor("gw2", [128, 128], BF16, kind="ExternalInput").ap()
    d_gb2 = nc.dram_tensor("gb2", [128, 1], F32, kind="ExternalInput").ap()
    d_w1s = nc.dram_tensor("w1s", [128, NSH, MLPD], BF16, kind="ExternalInput").ap()
    d_b1r = nc.dram_tensor("b1r", [BL, MLPD], F32, kind="ExternalInput").ap()
    d_w2r = nc.dram_tensor("w2r", [BL, MLPD], F32, kind="ExternalInput").ap()
    d_b2r = nc.dram_tensor("b2r", [BL, 1], F32, kind="ExternalInput").ap()
    d_out = nc.dram_tensor("out", [BL, 1], F32, kind="ExternalOutput").ap()

    with tile.TileContext(nc) as tc:
        with (
            tc.tile_pool(name="const", bufs=1) as const,
            tc.tile_pool(name="acts", bufs=1) as acts,
            tc.tile_pool(name="apool", bufs=10) as apool,
            tc.tile_pool(name="wpool", bufs=2) as wpool,
            tc.tile_pool(name="small", bufs=1) as small,
            tc.tile_pool(name="psum", bufs=7, space="PSUM") as psum,
            tc.tile_pool(name="psum2", bufs=1, space="PSUM") as psum2,
            tc.tile_pool(name="dram", bufs=1, space="DRAM") as dram,
        ):
            pools = dict(const=const, acts=acts, apool=apool, wpool=wpool,
                         small=small, psum=psum, psum2=psum2, dram=dram)

            # ---- load inputs to SBUF (per-sample x slices so conv can start early) ----
            xT_sb = const.tile([128, BL, H], BF16, name="xT_sb")
            for s in range(BL):
                nc.sync.dma_start(xT_sb[:, s, :], d_xT[s])
            wc_sb = const.tile([128, KS, 128], BF16, name="wc_sb")
            nc.sync.dma_start(wc_sb[:], d_wc.rearrange("k p o -> p k o"))
            cb_sb = const.tile([128, 1], F32, name="cb_sb")
            nc.sync.dma_start(cb_sb[:], d_cb[:])
            gw1_sb = const.tile([128, 128], BF16, name="gw1_sb")
            nc.sync.dma_start(gw1_sb[:], d_gw1[:])
            gb1_sb = const.tile([128, 1], F32, name="gb1_sb")
            nc.sync.dma_start(gb1_sb[:], d_gb1[:])
            gw2_sb = const.tile([128, 128], BF16, name="gw2_sb")
            nc.sync.dma_start(gw2_sb[:], d_gw2[:])
            gb2_sb = const.tile([128, 1], F32, name="gb2_sb")
            nc.sync.dma_start(gb2_sb[:], d_gb2[:])
            # bf16 A^T resident for the sample-outer layer-2 aggregation;
            # load overlaps conv + layer 1
            A_sb = const.tile([128, 16, N], BF16, name="A_sb")
            a_sb_dmas = []
            for q in range(4):
                a_sb_dmas.append(nc.sync.dma_start(
                    A_sb[:, 4 * q : 4 * q + 4, :],
                    d_at[4 * q : 4 * q + 4].rearrange("c p d -> p c d"),
                ))
            b1r_sb = small.tile([BL, MLPD], F32, name="b1r_sb")
            nc.sync.dma_start(b1r_sb[:], d_b1r[:])
            w2r_sb = small.tile([BL, MLPD], F32, name="w2r_sb")
            nc.sync.dma_start(w2r_sb[:], d_w2r[:])
            b2r_sb = small.tile([BL, 1], F32, name="b2r_sb")
            nc.sync.dma_start(b2r_sb[:], d_b2r[:])

            tensors = dict(
                xT_sb=xT_sb, wc_sb=wc_sb, cb_sb=cb_sb, at=d_at, at8=d_at8,
                A_sb=A_sb,
                gw1_sb=gw1_sb, gb1_sb=gb1_sb, gw2_sb=gw2_sb, gb2_sb=gb2_sb,
                w1s=d_w1s, b1r_sb=b1r_sb, w2r_sb=w2r_sb, b2r_sb=b2r_sb,
                n_warm=n_warm,
            )

            if loop_all_reps > 1:
                with tc.For_i(0, loop_all_reps, 1,
                              hint_engines=(mybir.EngineType.PE,)):
                    h2Ts = _emit_front(nc, tc, pools, tensors)
                    _emit_tail(nc, tc, pools, tensors, h2Ts, d_out,
                               collectives=False)
            elif front_reps == 1:
                h2Ts = _emit_front(nc, tc, pools, tensors)
                if "anchor_conv0" in tensors:
                    for d in a_sb_dmas:
                        add_dep_helper(d.ins, tensors["anchor_conv0"].ins,
                                       reason="delay A load past x load + conv start")
                for _ in range(tail_reps):
                    _emit_tail(nc, tc, pools, tensors, h2Ts, d_out,
                               collectives=collectives)
            else:
                with tc.For_i(0, front_reps, 1,
                              hint_engines=(mybir.EngineType.PE,)):
                    h2Ts = _emit_front(nc, tc, pools, tensors)
                for _ in range(tail_reps):
                    _emit_tail(nc, tc, pools, tensors, h2Ts, d_out,
                               collectives=collectives)

    nc.compile()
    return nc


def _prep_inputs(x, edge_index, conv_w, conv_b, gW1, gb1, gW2, gb2, W1, b1, W2, b2):
    """Host-side sharding / layout prep -> per-core input maps."""
    # gcn_norm (add_self_loops=True), duplicated edges accumulate
    src = np.concatenate([np.asarray(edge_index[0]), np.arange(N, dtype=np.int64)])
    dst = np.concatenate([np.asarray(edge_index[1]), np.arange(N, dtype=np.int64)])
    deg = np.bincount(dst, minlength=N).astype(np.float32)
    dinv = (1.0 / np.sqrt(np.maximum(deg, 1.0))).astype(np.float32)
    normv = dinv[src] * dinv[dst]
    AT = np.zeros((N, N), np.float32)
    np.add.at(AT, (src, dst), normv)
    at3 = np.ascontiguousarray(AT.reshape(16, 128, N))
    at_tiled = at3.astype(NP_BF16)
    at8_tiled = at3.astype(NP_FP8)

    wc = np.ascontiguousarray(
        np.asarray(conv_w)[:, 0, :, :].transpose(1, 2, 0)  # [KS, ic, oc]
    ).astype(NP_BF16)
    cb = np.asarray(conv_b, np.float32).reshape(128, 1)
    gw1 = np.asarray(gW1).astype(NP_BF16)
    gb1_ = np.asarray(gb1, np.float32).reshape(128, 1)
    gw2 = np.asarray(gW2).astype(NP_BF16)
    gb2_ = np.asarray(gb2, np.float32).reshape(128, 1)
    b1r = np.ascontiguousarray(np.broadcast_to(np.asarray(b1, np.float32), (BL, MLPD)))
    w2r = np.ascontiguousarray(np.broadcast_to(np.asarray(W2, np.float32)[:, 0], (BL, MLPD)))
    b2r = np.full((BL, 1), np.asarray(b2, np.float32)[0], np.float32)

    W1r = np.asarray(W1, np.float32).reshape(N, G2, MLPD)
    x_np = np.asarray(x, np.float32)

    in_maps = []
    for c in range(NCORES):
        xT = np.ascontiguousarray(
            x_np[c * BL : (c + 1) * BL].transpose(0, 2, 1)
        ).astype(NP_BF16)
        w1s = np.ascontiguousarray(
            W1r[c * NSH : (c + 1) * NSH].transpose(1, 0, 2)
        ).astype(NP_BF16)
        in_maps.append({
            "xT": xT, "at": at_tiled, "at8": at8_tiled, "wc": wc, "cb": cb,
            "gw1": gw1, "gb1": gb1_, "gw2": gw2, "gb2": gb2_,
            "w1s": w1s, "b1r": b1r, "w2r": w2r, "b2r": b2r,
        })
    return in_maps


_NC_CACHE = {}


def kernel(**inputs) -> np.ndarray:
    key = "full"
    if key not in _NC_CACHE:
        _NC_CACHE[key] = build_nc()
    nc = _NC_CACHE[key]
    in_maps = _prep_inputs(**inputs)
    res = run_bass_kernel_spmd(nc, in_maps, core_ids=list(range(NCORES)))
    out = np.concatenate([res.results[c]["out"] for c in range(NCORES)], axis=0)
    return out.astype(np.float32)



# revision 2
# speedup vs baseline: 1.0315x; 1.0315x over previous
"""Trainium2 Bass kernel for CNN+GCN+MLP (nn_CNNGCN_18236431139458).

Strategy (8 NeuronCores, one chip), v2:
  - Data-parallel over batch (4 samples/core) for conv + both GCN layers.
  - The scatter-aggregate is a dense matmul against the integer adjacency
    multiplicity matrix Adj^T (entries 0..3, EXACT in fp8), with the
    degree normalization dinv[src]*dinv[dst] factored out:
      pre-scale  by dinv[src]  -> folded into the PSUM->SBUF copy after
                                  each GCN linear (per-partition scale AP)
      post-scale by dinv[dst]  -> one vector multiply per PSUM tile
    One fp8 copy of Adj^T (4.2MB) serves both layers: layer 1 aggregates
    in fp8 DoubleRow (2x PE rate), layer 2 as bf16 lhsT x fp8 rhs (exact).
  - Conv runs fp8 DoubleRow with input-channel pairs packed on 64
    partitions (x and conv_w quantization noise dilutes through two
    rounds of graph averaging; measured ~0.5% final rel-l2).
  - MLP: W1 rows (nodes) sharded across cores as INTERLEAVED stripes
    (core j owns nodes {512k + 64j + i}), so each agg2 dst-block k is
    exactly one AllToAll chunk: the A2A for chunk k fires as soon as
    block k finishes, the per-chunk z matmuls overlap later agg2 blocks
    and A2A wire time, and the final ReduceScatter fires right after the
    last chunk instead of after a serialized A2A+MLP tail.
  - z accumulation uses 4-way PE column tiling (tile_position) so the
    M=32 matmuls run concurrently in disjoint 32-column groups.

Layouts (per core):
  x8p  [64 icp, 2, 4 s, 2050]    fp8 input, ic-pairs packed for DoubleRow
  adj8 [128 src, 16 scb, 2048]   fp8 Adj^T, resident
  h*T  [128 f, 4 s, 2048 n]      bf16 feature-major activations
  hw1  [128 n, 16 nch, 4s*128g]  fp8  = (h0@gW1) * dinv[src] * 64
  hw2  [128 n, 16 nch, 4s*128g]  bf16 = (h1@gW2) * dinv[src]
  w1c  [128 g, 64 n, 100]        bf16 W1 stripe chunk (streamed)
"""

import numpy as np
import ml_dtypes

import concourse.bass as bass
import concourse.mybir as mybir
import concourse.tile as tile
from concourse.tile import add_dep_helper
from concourse import bacc
from concourse.bass_utils import run_bass_kernel_spmd

BF16 = mybir.dt.bfloat16
FP8 = mybir.dt.float8e4
F32 = mybir.dt.float32
NP_BF16 = ml_dtypes.bfloat16
NP_FP8 = mybir.dt.np(FP8)

B, H, E = 32, 2050, 128
N = 2048
C = 128
G1 = G2 = 128
MLPD = 100
KS = 3
NE = 32768
NCORES = 8
BL = B // NCORES          # 4 samples per core
NSH = N // NCORES         # 256 nodes per core (interleaved stripes)
RG = [list(range(NCORES))]

Relu = mybir.ActivationFunctionType.Relu
Copy = mybir.ActivationFunctionType.Copy
DoubleRow = mybir.MatmulPerfMode.DoubleRow


def build_nc(num_devices=NCORES, collectives=True):
    nc = bacc.Bacc("TRN2", target_bir_lowering=False, debug=False,
                   num_devices=num_devices)

    d_x8p = nc.dram_tensor("x8p", [64, 2, BL, H], FP8, kind="ExternalInput").ap()
    d_adj8 = nc.dram_tensor("adj8", [16, 128, N], FP8, kind="ExternalInput").ap()
    d_wc8 = nc.dram_tensor("wc8", [64, 2, KS, 128], FP8, kind="ExternalInput").ap()
    d_cb = nc.dram_tensor("cb", [128, 1], F32, kind="ExternalInput").ap()
    d_gw1 = nc.dram_tensor("gw1", [128, 128], BF16, kind="ExternalInput").ap()
    d_gb1 = nc.dram_tensor("gb1", [128, 1], F32, kind="ExternalInput").ap()
    d_gw2 = nc.dram_tensor("gw2", [128, 128], BF16, kind="ExternalInput").ap()
    d_gb2 = nc.dram_tensor("gb2", [128, 1], F32, kind="ExternalInput").ap()
    d_dsc64 = nc.dram_tensor("dsc64", [128, 16], F32, kind="ExternalInput").ap()
    d_dsc1 = nc.dram_tensor("dsc1", [128, 16], F32, kind="ExternalInput").ap()
    d_dinvrow = nc.dram_tensor("dinvrow", [128, N], F32, kind="ExternalInput").ap()
    d_w1s = nc.dram_tensor("w1s", [128, NSH, MLPD], BF16, kind="ExternalInput").ap()
    d_b1r = nc.dram_tensor("b1r", [BL, MLPD], F32, kind="ExternalInput").ap()
    d_w2r = nc.dram_tensor("w2r", [BL, MLPD], F32, kind="ExternalInput").ap()
    d_b2r = nc.dram_tensor("b2r", [BL, 1], F32, kind="ExternalInput").ap()
    d_out = nc.dram_tensor("out", [BL, 1], F32, kind="ExternalOutput").ap()

    with tile.TileContext(nc) as tc:
        with (
            tc.tile_pool(name="const", bufs=1) as const,
            tc.tile_pool(name="acts", bufs=1) as acts,
            tc.tile_pool(name="wpool", bufs=2) as wpool,
            tc.tile_pool(name="small", bufs=1) as small,
            tc.tile_pool(name="psum", bufs=7, space="PSUM") as psum,
            tc.tile_pool(name="psum2", bufs=1, space="PSUM") as psum2,
            tc.tile_pool(name="dram", bufs=1, space="DRAM") as dram,
        ):
            # ---- input loads (x first so conv starts early, then Adj) ----
            x8_sb = const.tile([64, 2, BL, H], FP8, name="x8_sb")
            for s in range(BL):
                nc.sync.dma_start(x8_sb[:, :, s, :], d_x8p[:, :, s, :])
            wc8_sb = const.tile([64, 2, KS, 128], FP8, name="wc8_sb")
            nc.sync.dma_start(wc8_sb[:], d_wc8[:])
            cb_sb = const.tile([128, 1], F32, name="cb_sb")
            nc.sync.dma_start(cb_sb[:], d_cb[:])
            gw1_sb = const.tile([128, 128], BF16, name="gw1_sb")
            nc.sync.dma_start(gw1_sb[:], d_gw1[:])
            gb1_sb = const.tile([128, 1], F32, name="gb1_sb")
            nc.sync.dma_start(gb1_sb[:], d_gb1[:])
            gw2_sb = const.tile([128, 128], BF16, name="gw2_sb")
            nc.sync.dma_start(gw2_sb[:], d_gw2[:])
            gb2_sb = const.tile([128, 1], F32, name="gb2_sb")
            nc.sync.dma_start(gb2_sb[:], d_gb2[:])
            dsc64_sb = const.tile([128, 16], F32, name="dsc64_sb")
            nc.sync.dma_start(dsc64_sb[:], d_dsc64[:])
            dsc1_sb = const.tile([128, 16], F32, name="dsc1_sb")
            nc.sync.dma_start(dsc1_sb[:], d_dsc1[:])
            adj8_sb = const.tile([128, 16, N], FP8, name="adj8_sb")
            for q in range(4):
                nc.sync.dma_start(
                    adj8_sb[:, 4 * q : 4 * q + 4, :],
                    d_adj8[4 * q : 4 * q + 4].rearrange("c p d -> p c d"),
                )
            dinvrow_sb = const.tile([128, N], F32, name="dinvrow_sb")
            nc.sync.dma_start(dinvrow_sb[:], d_dinvrow[:])
            b1r_sb = small.tile([BL, MLPD], F32, name="b1r_sb")
            nc.sync.dma_start(b1r_sb[:], d_b1r[:])
            w2r_sb = small.tile([BL, MLPD], F32, name="w2r_sb")
            nc.sync.dma_start(w2r_sb[:], d_w2r[:])
            b2r_sb = small.tile([BL, 1], F32, name="b2r_sb")
            nc.sync.dma_start(b2r_sb[:], d_b2r[:])

            # ---- conv (fp8 DoubleRow, ic-pairs) + lin1 interleaved ----
            h0T = acts.tile([128, BL, N], BF16, tag="hT", bufs=2, name="h0T")
            hw1 = acts.tile([128, 16, BL * 128], FP8, tag="hw1", name="hw1")
            anchor_conv0 = None
            for nt in range(4):
                for s in range(BL):
                    ps = psum.tile([128, 512], F32, tag="ps", name="ps_conv")
                    for k in range(KS):
                        nc.tensor.matmul(
                            ps[:],
                            lhsT=wc8_sb[:, :, k, :],
                            rhs=x8_sb[:, :, s, nt * 512 + k : nt * 512 + k + 512],
                            start=(k == 0),
                            stop=(k == KS - 1),
                            perf_mode=DoubleRow,
                        )
                    act = nc.scalar.activation(
                        h0T[:, s, nt * 512 : (nt + 1) * 512], ps[:], Relu,
                        bias=cb_sb[:], scale=1.0 / 64.0)
                    if anchor_conv0 is None:
                        anchor_conv0 = act
                for nch in range(4 * nt, 4 * nt + 4):
                    ps = psum.tile([128, 512], F32, tag="ps", name="ps_lin1")
                    for s in range(BL):
                        nc.tensor.matmul(
                            ps[:, s * 128 : (s + 1) * 128],
                            lhsT=h0T[:, s, nch * 128 : (nch + 1) * 128],
                            rhs=gw1_sb[:],
                            start=True,
                            stop=True,
                        )
                    nc.scalar.activation(hw1[:, nch, :], ps[:], Copy,
                                         scale=dsc64_sb[:, nch : nch + 1])

            # ---- agg1 (fp8 DoubleRow vs Adj) + lin2 interleaved ----
            h1T = acts.tile([128, BL, N], BF16, tag="hT", bufs=2, name="h1T")
            hw2 = acts.tile([128, 16, BL * 128], BF16, tag="hw2", name="hw2")
            anchor_agg1 = None
            for dt in range(4):
                pss = [psum.tile([128, 512], F32, tag="ps", name=f"ps_a1_{s}")
                       for s in range(BL)]
                for a2 in range(8):
                    for s in range(BL):
                        nc.tensor.matmul(
                            pss[s][:],
                            lhsT=hw1[:, 2 * a2 : 2 * a2 + 2, s * 128 : (s + 1) * 128],
                            rhs=adj8_sb[:, 2 * a2 : 2 * a2 + 2, dt * 512 : (dt + 1) * 512],
                            start=(a2 == 0),
                            stop=(a2 == 7),
                            perf_mode=DoubleRow,
                        )
                for s in range(BL):
                    nc.vector.tensor_mul(pss[s][:], pss[s][:],
                                         dinvrow_sb[:, dt * 512 : (dt + 1) * 512])
                    act = nc.scalar.activation(
                        h1T[:, s, dt * 512 : (dt + 1) * 512], pss[s][:], Relu,
                        bias=gb1_sb[:], scale=1.0 / 64.0)
                    if anchor_agg1 is None:
                        anchor_agg1 = act
                for nch in range(4 * dt, 4 * dt + 4):
                    ps = psum.tile([128, 512], F32, tag="ps", name="ps_lin2")
                    for s in range(BL):
                        nc.tensor.matmul(
                            ps[:, s * 128 : (s + 1) * 128],
                            lhsT=h1T[:, s, nch * 128 : (nch + 1) * 128],
                            rhs=gw2_sb[:],
                            start=True,
                            stop=True,
                        )
                    nc.scalar.activation(hw2[:, nch, :], ps[:], Copy,
                                         scale=dsc1_sb[:, nch : nch + 1])

            # ---- agg2 chunks + per-chunk A2A + overlapped MLP z ----
            psz = psum2.tile([128, MLPD], F32, tag="psz", name="psz")
            w1cs, h2aks = [], []

            def emit_agg2_chunk(k):
                # W1 stripe chunk prefetch (ring bufs=2 keeps it just ahead)
                w1c = wpool.tile([128, 64, MLPD], BF16, tag="w1c", bufs=2,
                                 name=f"w1c{k}")
                dma = nc.sync.dma_start(w1c[:], d_w1s[:, k * 64 : (k + 1) * 64, :])
                if anchor_conv0 is not None:
                    add_dep_helper(dma.ins, anchor_conv0.ins,
                                   reason="delay W1 prefetch past x/adj loads")
                w1cs.append(w1c)

                pss = [psum.tile([128, 512], F32, tag="ps", name=f"ps_a2_{s}")
                       for s in range(BL)]
                for sc in range(16):
                    for s in range(BL):
                        nc.tensor.matmul(
                            pss[s][:],
                            lhsT=hw2[:, sc, s * 128 : (s + 1) * 128],
                            rhs=adj8_sb[:, sc, k * 512 : (k + 1) * 512],
                            start=(sc == 0),
                            stop=(sc == 15),
                        )
                h2Tk = acts.tile([128, BL, 512], BF16, tag="h2T", bufs=2,
                                 name=f"h2T{k}")
                for s in range(BL):
                    nc.vector.tensor_mul(pss[s][:], pss[s][:],
                                         dinvrow_sb[:, k * 512 : (k + 1) * 512])
                    nc.scalar.activation(h2Tk[:, s, :], pss[s][:], Relu,
                                         bias=gb2_sb[:])
                # stage + AllToAll for this chunk
                a2a_in = dram.tile([NCORES, 128, BL, 64], BF16,
                                   tag=f"a2a_in{k}", name=f"a2a_in{k}")
                a2a_out = dram.tile([NCORES, 128, BL, 64], BF16,
                                    tag=f"a2a_out{k}", name=f"a2a_out{k}")
                for j in range(NCORES):
                    nc.sync.dma_start(a2a_in[j], h2Tk[:, :, 64 * j : 64 * j + 64])
                h2ak = acts.tile([128, B, 64], BF16, tag="h2a", bufs=2,
                                 name=f"h2a{k}")
                if collectives:
                    nc.gpsimd.collective_compute(
                        "AllToAll", mybir.AluOpType.bypass, replica_groups=RG,
                        ins=[a2a_in.opt()], outs=[a2a_out.opt()],
                    )
                    for j in range(NCORES):
                        nc.sync.dma_start(h2ak[:, j * BL : (j + 1) * BL, :],
                                          a2a_out[j])
                else:
                    for j in range(NCORES):
                        nc.sync.dma_start(h2ak[:, j * BL : (j + 1) * BL, :],
                                          a2a_in[j])
                h2aks.append(h2ak)

            def emit_z_chunk(k):
                h2ak, w1c = h2aks[k], w1cs[k]
                for i in range(64):
                    n = k * 64 + i
                    j2 = i % 4
                    nc.tensor.matmul(
                        psz[32 * j2 : 32 * (j2 + 1), :],
                        lhsT=h2ak[:, :, i],
                        rhs=w1c[:, i, :],
                        start=(n < 4),
                        stop=(n >= NSH - 4),
                        tile_position=(0, 32 * j2),
                    )

            emit_agg2_chunk(0)
            emit_agg2_chunk(1)
            emit_z_chunk(0)
            emit_agg2_chunk(2)
            emit_z_chunk(1)
            emit_agg2_chunk(3)
            emit_z_chunk(2)
            emit_z_chunk(3)

            # ---- combine column groups, ReduceScatter, local MLP tail ----
            z_sb = small.tile([32, MLPD], F32, tag="z_sb", name="z_sb")
            nc.vector.tensor_copy(z_sb[:], psz[0:32, :])
            for j2 in range(1, 4):
                nc.vector.tensor_add(z_sb[:], z_sb[:],
                                     psz[32 * j2 : 32 * (j2 + 1), :])
            rs_in = dram.tile([32, MLPD], F32, tag="rs_in", name="rs_in")
            rs_out = dram.tile([BL, MLPD], F32, tag="rs_out", name="rs_out")
            nc.sync.dma_start(rs_in[:], z_sb[:])
            zloc = small.tile([BL, MLPD], F32, tag="zloc", name="zloc")
            if collectives:
                nc.gpsimd.collective_compute(
                    "ReduceScatter", mybir.AluOpType.add, replica_groups=RG,
                    ins=[rs_in.opt()], outs=[rs_out.opt()],
                )
                nc.sync.dma_start(zloc[:], rs_out[:])
            else:
                nc.sync.dma_start(zloc[:], rs_in[0:BL, :])
            hm = small.tile([BL, MLPD], F32, tag="hm", name="hm")
            nc.vector.tensor_add(hm[:], zloc[:], b1r_sb[:])
            nc.vector.tensor_scalar_max(hm[:], hm[:], 0.0)
            nc.vector.tensor_mul(hm[:], hm[:], w2r_sb[:])
            osb = small.tile([BL, 1], F32, tag="osb", name="osb")
            nc.vector.reduce_sum(osb[:], hm[:], axis=mybir.AxisListType.X)
            nc.vector.tensor_add(osb[:], osb[:], b2r_sb[:])
            nc.sync.dma_start(d_out[:], osb[:])

    nc.compile()
    return nc


def _prep_inputs(x, edge_index, conv_w, conv_b, gW1, gb1, gW2, gb2, W1, b1, W2, b2):
    """Host-side sharding / layout prep -> per-core input maps."""
    # gcn_norm (add_self_loops=True); Adj^T holds integer multiplicities
    src = np.concatenate([np.asarray(edge_index[0]), np.arange(N, dtype=np.int64)])
    dst = np.concatenate([np.asarray(edge_index[1]), np.arange(N, dtype=np.int64)])
    deg = np.bincount(dst, minlength=N).astype(np.float32)
    dinv = (1.0 / np.sqrt(np.maximum(deg, 1.0))).astype(np.float32)
    AdjT = np.zeros((N, N), np.float32)
    np.add.at(AdjT, (src, dst), np.ones_like(src, dtype=np.float32))
    adj8 = np.ascontiguousarray(AdjT.reshape(16, 128, N)).astype(NP_FP8)

    # conv weights: ic-pairs on 64 partitions, x64 range centering
    wc = np.asarray(conv_w, np.float32)[:, 0, :, :]          # [oc, KS, ic]
    wc8 = np.ascontiguousarray(
        (wc * 64.0).transpose(2, 1, 0).reshape(64, 2, KS, 128)
    ).astype(NP_FP8)
    cb = np.asarray(conv_b, np.float32).reshape(128, 1)
    gw1 = np.asarray(gW1).astype(NP_BF16)
    gb1_ = np.asarray(gb1, np.float32).reshape(128, 1)
    gw2 = np.asarray(gW2).astype(NP_BF16)
    gb2_ = np.asarray(gb2, np.float32).reshape(128, 1)
    dsc64 = np.ascontiguousarray((dinv * 64.0).reshape(16, 128).T)
    dsc1 = np.ascontiguousarray(dinv.reshape(16, 128).T)
    dinvrow = np.ascontiguousarray(np.broadcast_to(dinv[None, :], (128, N)))
    b1r = np.ascontiguousarray(np.broadcast_to(np.asarray(b1, np.float32), (BL, MLPD)))
    w2r = np.ascontiguousarray(np.broadcast_to(np.asarray(W2, np.float32)[:, 0], (BL, MLPD)))
    b2r = np.full((BL, 1), np.asarray(b2, np.float32)[0], np.float32)

    W1r = np.asarray(W1, np.float32).reshape(N, G2, MLPD)
    x_np = np.asarray(x, np.float32)

    in_maps = []
    for c in range(NCORES):
        # x8p[p, j, s, n] = x[c*BL+s, n, 2p+j]
        xs = x_np[c * BL : (c + 1) * BL]                     # [BL, H, E]
        x8p = np.ascontiguousarray(
            xs.transpose(2, 0, 1).reshape(64, 2, BL, H)
        ).astype(NP_FP8)
        # interleaved stripe shard: node(k, i) = 512*k + 64*c + i
        nodes = (512 * np.arange(4)[:, None] + 64 * c
                 + np.arange(64)[None, :]).reshape(-1)
        w1s = np.ascontiguousarray(W1r[nodes].transpose(1, 0, 2)).astype(NP_BF16)
        in_maps.append({
            "x8p": x8p, "adj8": adj8, "wc8": wc8, "cb": cb,
            "gw1": gw1, "gb1": gb1_, "gw2": gw2, "gb2": gb2_,
            "dsc64": dsc64, "dsc1": dsc1, "dinvrow": dinvrow,
            "w1s": w1s, "b1r": b1r, "w2r": w2r, "b2r": b2r,
        })
    return in_maps


_NC_CACHE = {}


def kernel(**inputs) -> np.ndarray:
    key = "full"
    if key not in _NC_CACHE:
        _NC_CACHE[key] = build_nc()
    nc = _NC_CACHE[key]
    in_maps = _prep_inputs(**inputs)
    res = run_bass_kernel_spmd(nc, in_maps, core_ids=list(range(NCORES)))
    out = np.concatenate([res.results[c]["out"] for c in range(NCORES)], axis=0)
    return out.astype(np.float32)


# revision 14
# speedup vs baseline: 1.0711x; 1.0385x over previous
"""Trainium2 Bass kernel for CNN+GCN+MLP (nn_CNNGCN_18236431139458).

Strategy (8 NeuronCores, one chip), v2:
  - Data-parallel over batch (4 samples/core) for conv + both GCN layers.
  - The scatter-aggregate is a dense matmul against the integer adjacency
    multiplicity matrix Adj^T (entries 0..3, EXACT in fp8), with the
    degree normalization dinv[src]*dinv[dst] factored out:
      pre-scale  by dinv[src]  -> folded into the PSUM->SBUF copy after
                                  each GCN linear (per-partition scale AP)
      post-scale by dinv[dst]  -> one vector multiply per PSUM tile
    One fp8 copy of Adj^T (4.2MB) serves both layers; both aggregations
    run fp8 DoubleRow (2x PE rate). The aggregated h (post-relu, bf16)
    and the MLP stay high precision, so the fp8 activation quantization
    noise (iid per element) averages away through the graph aggregation
    while weight-side noise (which would NOT average) is avoided.
  - Conv runs fp8 DoubleRow with input-channel pairs packed on 64
    partitions (x and conv_w quantization noise dilutes through two
    rounds of graph averaging; measured ~0.5% final rel-l2).
  - MLP: W1 rows (nodes) sharded across cores as INTERLEAVED stripes
    (core j owns nodes {512k + 64j + i}), so each agg2 dst-block k is
    exactly one AllToAll chunk: the A2A for chunk k fires as soon as
    block k finishes, the per-chunk z matmuls overlap later agg2 blocks
    and A2A wire time, and the final ReduceScatter fires right after the
    last chunk instead of after a serialized A2A+MLP tail.
  - z accumulation uses 4-way PE column tiling (tile_position) so the
    M=32 matmuls run concurrently in disjoint 32-column groups.

Layouts (per core):
  x8p  [64 icp, 2, 4 s, 2050]    fp8 input, ic-pairs packed for DoubleRow
  adj8 [128 src, 16 scb, 2048]   fp8 Adj^T, resident
  h*T  [128 f, 4 s, 2048 n]      bf16 feature-major activations
  hw1  [128 n, 16 nch, 4s*128g]  fp8  = (h0@gW1) * dinv[src] * 64
  hw2  [128 n, 16 nch, 4s*128g]  fp8  = (h1@gW2) * dinv[src] * 256
  w1c  [128 g, 64 n, 100]        bf16 W1 stripe chunks (all resident)
"""

import numpy as np
import ml_dtypes

import concourse.bass as bass
import concourse.mybir as mybir
import concourse.tile as tile
from concourse.tile import add_dep_helper
from concourse import bacc
from concourse.bass_utils import run_bass_kernel_spmd

BF16 = mybir.dt.bfloat16
FP8 = mybir.dt.float8e4
F32 = mybir.dt.float32
NP_BF16 = ml_dtypes.bfloat16
NP_FP8 = mybir.dt.np(FP8)

B, H, E = 32, 2050, 128
N = 2048
C = 128
G1 = G2 = 128
MLPD = 100
KS = 3
NE = 32768
NCORES = 8
BL = B // NCORES          # 4 samples per core
NSH = N // NCORES         # 256 nodes per core (interleaved stripes)
RG = [list(range(NCORES))]

Relu = mybir.ActivationFunctionType.Relu
Copy = mybir.ActivationFunctionType.Copy
DoubleRow = mybir.MatmulPerfMode.DoubleRow


def build_nc(num_devices=NCORES, collectives=True):
    nc = bacc.Bacc("TRN2", target_bir_lowering=False, debug=False,
                   num_devices=num_devices)

    d_x8p = nc.dram_tensor("x8p", [64, 2, BL, H], FP8, kind="ExternalInput").ap()
    d_adj8 = nc.dram_tensor("adj8", [16, 128, N], FP8, kind="ExternalInput").ap()
    d_wc8 = nc.dram_tensor("wc8", [64, 2, KS, 128], FP8, kind="ExternalInput").ap()
    d_cb = nc.dram_tensor("cb", [128, 1], F32, kind="ExternalInput").ap()
    d_gw1 = nc.dram_tensor("gw1", [128, 128], BF16, kind="ExternalInput").ap()
    d_gb1 = nc.dram_tensor("gb1", [128, 1], F32, kind="ExternalInput").ap()
    d_gw2 = nc.dram_tensor("gw2", [128, 128], BF16, kind="ExternalInput").ap()
    d_gb2 = nc.dram_tensor("gb2", [128, 1], F32, kind="ExternalInput").ap()
    d_dsc64 = nc.dram_tensor("dsc64", [128, 16], F32, kind="ExternalInput").ap()
    d_dsc256 = nc.dram_tensor("dsc256", [128, 16], F32, kind="ExternalInput").ap()
    d_dinvrow = nc.dram_tensor("dinvrow", [128, N], F32, kind="ExternalInput").ap()
    d_w1s = nc.dram_tensor("w1s", [128, NSH, MLPD], BF16, kind="ExternalInput").ap()
    d_b1r = nc.dram_tensor("b1r", [BL, MLPD], F32, kind="ExternalInput").ap()
    d_w2r = nc.dram_tensor("w2r", [BL, MLPD], F32, kind="ExternalInput").ap()
    d_b2r = nc.dram_tensor("b2r", [BL, 1], F32, kind="ExternalInput").ap()
    d_out = nc.dram_tensor("out", [BL, 1], F32, kind="ExternalOutput").ap()

    with tile.TileContext(nc) as tc:
        with (
            tc.tile_pool(name="const", bufs=1) as const,
            tc.tile_pool(name="acts", bufs=1) as acts,
            tc.tile_pool(name="wpool", bufs=2) as wpool,
            tc.tile_pool(name="small", bufs=1) as small,
            tc.tile_pool(name="psum", bufs=7, space="PSUM") as psum,
            tc.tile_pool(name="psum2", bufs=1, space="PSUM") as psum2,
            tc.tile_pool(name="dram", bufs=1, space="DRAM") as dram,
        ):
            # ---- input loads (x first so conv starts early, then Adj) ----
            x8_sb = const.tile([64, 2, BL, H], FP8, name="x8_sb")
            for s in range(BL):
                nc.sync.dma_start(x8_sb[:, :, s, :], d_x8p[:, :, s, :])
            wc8_sb = const.tile([64, 2, KS, 128], FP8, name="wc8_sb")
            nc.sync.dma_start(wc8_sb[:], d_wc8[:])
            cb_sb = const.tile([128, 1], F32, name="cb_sb")
            nc.sync.dma_start(cb_sb[:], d_cb[:])
            gw1_sb = const.tile([128, 128], BF16, name="gw1_sb")
            nc.sync.dma_start(gw1_sb[:], d_gw1[:])
            gb1_sb = const.tile([128, 1], F32, name="gb1_sb")
            nc.sync.dma_start(gb1_sb[:], d_gb1[:])
            gw2_sb = const.tile([128, 128], BF16, name="gw2_sb")
            nc.sync.dma_start(gw2_sb[:], d_gw2[:])
            gb2_sb = const.tile([128, 1], F32, name="gb2_sb")
            nc.sync.dma_start(gb2_sb[:], d_gb2[:])
            dsc64_sb = const.tile([128, 16], F32, name="dsc64_sb")
            nc.sync.dma_start(dsc64_sb[:], d_dsc64[:])
            dsc256_sb = const.tile([128, 16], F32, name="dsc256_sb")
            nc.sync.dma_start(dsc256_sb[:], d_dsc256[:])
            adj8_sb = const.tile([128, 16, N], FP8, name="adj8_sb")
            for q in range(4):
                nc.sync.dma_start(
                    adj8_sb[:, 4 * q : 4 * q + 4, :],
                    d_adj8[4 * q : 4 * q + 4].rearrange("c p d -> p c d"),
                )
            dinvrow_sb = const.tile([128, N], F32, name="dinvrow_sb")
            nc.sync.dma_start(dinvrow_sb[:], d_dinvrow[:])
            b1r_sb = small.tile([BL, MLPD], F32, name="b1r_sb")
            nc.sync.dma_start(b1r_sb[:], d_b1r[:])
            w2r_sb = small.tile([BL, MLPD], F32, name="w2r_sb")
            nc.sync.dma_start(w2r_sb[:], d_w2r[:])
            b2r_sb = small.tile([BL, 1], F32, name="b2r_sb")
            nc.sync.dma_start(b2r_sb[:], d_b2r[:])

            # ---- conv (fp8 DoubleRow, ic-pairs) + lin1 interleaved ----
            h0T = acts.tile([128, BL, N], BF16, tag="hT", bufs=2, name="h0T")
            hw1 = acts.tile([128, 16, BL * 128], FP8, tag="hw1", name="hw1")
            anchor_conv0 = None
            for nt in range(4):
                for s in range(BL):
                    ps = psum.tile([128, 512], F32, tag="ps", name="ps_conv")
                    for k in range(KS):
                        nc.tensor.matmul(
                            ps[:],
                            lhsT=wc8_sb[:, :, k, :],
                            rhs=x8_sb[:, :, s, nt * 512 + k : nt * 512 + k + 512],
                            start=(k == 0),
                            stop=(k == KS - 1),
                            perf_mode=DoubleRow,
                        )
                    act = nc.scalar.activation(
                        h0T[:, s, nt * 512 : (nt + 1) * 512], ps[:], Relu,
                        bias=cb_sb[:], scale=1.0 / 64.0)
                    if anchor_conv0 is None:
                        anchor_conv0 = act
                        # W1 stripe loads: whole shard resident; start after
                        # the x/adj queue so they don't fight the front
                        w1cs = []
                        for kk in range(4):
                            w1c = wpool.tile([128, 64, MLPD], BF16, tag=f"w1c{kk}",
                                             bufs=1, name=f"w1c{kk}")
                            dma = nc.sync.dma_start(
                                w1c[:], d_w1s[:, kk * 64 : (kk + 1) * 64, :])
                            add_dep_helper(dma.ins, anchor_conv0.ins,
                                           reason="delay W1 loads past x/adj loads")
                            w1cs.append(w1c)
                for nch in range(4 * nt, 4 * nt + 4):
                    ps = psum.tile([128, 512], F32, tag="ps", name="ps_lin1")
                    for s in range(BL):
                        nc.tensor.matmul(
                            ps[:, s * 128 : (s + 1) * 128],
                            lhsT=h0T[:, s, nch * 128 : (nch + 1) * 128],
                            rhs=gw1_sb[:],
                            start=True,
                            stop=True,
                        )
                    nc.scalar.activation(hw1[:, nch, :], ps[:], Copy,
                                         scale=dsc64_sb[:, nch : nch + 1])

            # ---- agg1 (fp8 DoubleRow vs Adj) + lin2 interleaved ----
            # s-outer so each sample's PSUM bank drains (vector+scalar) while
            # the next sample's matmuls run -> no bank-starvation stalls
            h1T = acts.tile([128, BL, N], BF16, tag="hT", bufs=2, name="h1T")
            hw2 = acts.tile([128, 16, BL * 128], FP8, tag="hw2", name="hw2")
            for dt in range(4):
                for s in range(BL):
                    ps = psum.tile([128, 512], F32, tag="ps", name=f"ps_a1_{s}")
                    for a2 in range(8):
                        nc.tensor.matmul(
                            ps[:],
                            lhsT=hw1[:, 2 * a2 : 2 * a2 + 2, s * 128 : (s + 1) * 128],
                            rhs=adj8_sb[:, 2 * a2 : 2 * a2 + 2, dt * 512 : (dt + 1) * 512],
                            start=(a2 == 0),
                            stop=(a2 == 7),
                            perf_mode=DoubleRow,
                        )
                    nc.vector.tensor_mul(ps[:], ps[:],
                                         dinvrow_sb[:, dt * 512 : (dt + 1) * 512])
                    nc.scalar.activation(
                        h1T[:, s, dt * 512 : (dt + 1) * 512], ps[:], Relu,
                        bias=gb1_sb[:], scale=1.0 / 64.0)
                for nch in range(4 * dt, 4 * dt + 4):
                    ps = psum.tile([128, 512], F32, tag="ps", name="ps_lin2")
                    for s in range(BL):
                        nc.tensor.matmul(
                            ps[:, s * 128 : (s + 1) * 128],
                            lhsT=h1T[:, s, nch * 128 : (nch + 1) * 128],
                            rhs=gw2_sb[:],
                            start=True,
                            stop=True,
                        )
                    nc.scalar.activation(hw2[:, nch, :], ps[:], Copy,
                                         scale=dsc256_sb[:, nch : nch + 1])

            # ---- agg2 chunks (fp8 DoubleRow) + per-chunk A2A + MLP z ----
            psz = psum2.tile([128, MLPD], F32, tag="psz", name="psz")
            h2aks = []

            def emit_agg2_chunk(k):
                h2Tk = acts.tile([128, BL, 512], BF16, tag="h2T", bufs=2,
                                 name=f"h2T{k}")
                for s in range(BL):
                    ps = psum.tile([128, 512], F32, tag="ps", name=f"ps_a2_{s}")
                    for a2 in range(8):
                        nc.tensor.matmul(
                            ps[:],
                            lhsT=hw2[:, 2 * a2 : 2 * a2 + 2, s * 128 : (s + 1) * 128],
                            rhs=adj8_sb[:, 2 * a2 : 2 * a2 + 2, k * 512 : (k + 1) * 512],
                            start=(a2 == 0),
                            stop=(a2 == 7),
                            perf_mode=DoubleRow,
                        )
                    nc.vector.tensor_mul(ps[:], ps[:],
                                         dinvrow_sb[:, k * 512 : (k + 1) * 512])
                    nc.scalar.activation(h2Tk[:, s, :], ps[:], Relu,
                                         bias=gb2_sb[:], scale=1.0 / 256.0)
                # stage + AllToAll for this chunk (single rearranged DMAs)
                a2a_in = dram.tile([NCORES, 128, BL, 64], BF16,
                                   tag=f"a2a_in{k}", name=f"a2a_in{k}")
                a2a_out = dram.tile([NCORES, 128, BL, 64], BF16,
                                    tag=f"a2a_out{k}", name=f"a2a_out{k}")
                nc.sync.dma_start(
                    a2a_in[:],
                    h2Tk[:].rearrange("p s (j n) -> j p s n", j=NCORES))
                h2ak = acts.tile([128, NCORES, BL, 64], BF16, tag="h2a", bufs=2,
                                 name=f"h2a{k}")
                if collectives:
                    nc.gpsimd.collective_compute(
                        "AllToAll", mybir.AluOpType.bypass, replica_groups=RG,
                        ins=[a2a_in.opt()], outs=[a2a_out.opt()],
                    )
                    nc.sync.dma_start(
                        h2ak[:], a2a_out[:].rearrange("j p s n -> p j s n"))
                else:
                    nc.sync.dma_start(
                        h2ak[:], a2a_in[:].rearrange("j p s n -> p j s n"))
                h2aks.append(h2ak)

            def emit_z_chunk(k):
                h2ak, w1c = h2aks[k], w1cs[k]
                for i in range(64):
                    n = k * 64 + i
                    j2 = i % 4
                    nc.tensor.matmul(
                        psz[32 * j2 : 32 * (j2 + 1), :],
                        lhsT=h2ak[:, :, :, i],
                        rhs=w1c[:, i, :],
                        start=(n < 4),
                        stop=(n >= NSH - 4),
                        tile_position=(0, 32 * j2),
                    )

            emit_agg2_chunk(0)
            emit_agg2_chunk(1)
            emit_z_chunk(0)
            emit_agg2_chunk(2)
            emit_z_chunk(1)
            emit_agg2_chunk(3)
            emit_z_chunk(2)
            emit_z_chunk(3)

            # ---- combine column groups, ReduceScatter, local MLP tail ----
            z_sb = small.tile([32, MLPD], F32, tag="z_sb", name="z_sb")
            nc.vector.tensor_copy(z_sb[:], psz[0:32, :])
            for j2 in range(1, 4):
                nc.vector.tensor_add(z_sb[:], z_sb[:],
                                     psz[32 * j2 : 32 * (j2 + 1), :])
            rs_in = dram.tile([32, MLPD], F32, tag="rs_in", name="rs_in")
            rs_out = dram.tile([BL, MLPD], F32, tag="rs_out", name="rs_out")
            nc.sync.dma_start(rs_in[:], z_sb[:])
            zloc = small.tile([BL, MLPD], F32, tag="zloc", name="zloc")
            if collectives:
                nc.gpsimd.collective_compute(
                    "ReduceScatter", mybir.AluOpType.add, replica_groups=RG,
                    ins=[rs_in.opt()], outs=[rs_out.opt()],
                )
                nc.sync.dma_start(zloc[:], rs_out[:])
            else:
                nc.sync.dma_start(zloc[:], rs_in[0:BL, :])
            hm = small.tile([BL, MLPD], F32, tag="hm", name="hm")
            nc.vector.tensor_add(hm[:], zloc[:], b1r_sb[:])
            nc.vector.tensor_scalar_max(hm[:], hm[:], 0.0)
            nc.vector.tensor_mul(hm[:], hm[:], w2r_sb[:])
            osb = small.tile([BL, 1], F32, tag="osb", name="osb")
            nc.vector.reduce_sum(osb[:], hm[:], axis=mybir.AxisListType.X)
            nc.vector.tensor_add(osb[:], osb[:], b2r_sb[:])
            nc.sync.dma_start(d_out[:], osb[:])

    nc.compile()
    return nc


def _prep_inputs(x, edge_index, conv_w, conv_b, gW1, gb1, gW2, gb2, W1, b1, W2, b2):
    """Host-side sharding / layout prep -> per-core input maps."""
    # gcn_norm (add_self_loops=True); Adj^T holds integer multiplicities
    src = np.concatenate([np.asarray(edge_index[0]), np.arange(N, dtype=np.int64)])
    dst = np.concatenate([np.asarray(edge_index[1]), np.arange(N, dtype=np.int64)])
    deg = np.bincount(dst, minlength=N).astype(np.float32)
    dinv = (1.0 / np.sqrt(np.maximum(deg, 1.0))).astype(np.float32)
    AdjT = np.zeros((N, N), np.float32)
    np.add.at(AdjT, (src, dst), np.ones_like(src, dtype=np.float32))
    adj8 = np.ascontiguousarray(AdjT.reshape(16, 128, N)).astype(NP_FP8)

    # conv weights: ic-pairs on 64 partitions, x64 range centering
    wc = np.asarray(conv_w, np.float32)[:, 0, :, :]          # [oc, KS, ic]
    wc8 = np.ascontiguousarray(
        (wc * 64.0).transpose(2, 1, 0).reshape(64, 2, KS, 128)
    ).astype(NP_FP8)
    cb = np.asarray(conv_b, np.float32).reshape(128, 1)
    gw1 = np.asarray(gW1).astype(NP_BF16)
    gb1_ = np.asarray(gb1, np.float32).reshape(128, 1)
    gw2 = np.asarray(gW2).astype(NP_BF16)
    gb2_ = np.asarray(gb2, np.float32).reshape(128, 1)
    dsc64 = np.ascontiguousarray((dinv * 64.0).reshape(16, 128).T)
    dsc256 = np.ascontiguousarray((dinv * 256.0).reshape(16, 128).T)
    dinvrow = np.ascontiguousarray(np.broadcast_to(dinv[None, :], (128, N)))
    b1r = np.ascontiguousarray(np.broadcast_to(np.asarray(b1, np.float32), (BL, MLPD)))
    w2r = np.ascontiguousarray(np.broadcast_to(np.asarray(W2, np.float32)[:, 0], (BL, MLPD)))
    b2r = np.full((BL, 1), np.asarray(b2, np.float32)[0], np.float32)

    W1r = np.asarray(W1, np.float32).reshape(N, G2, MLPD)
    x_np = np.asarray(x, np.float32)

    in_maps = []
    for c in range(NCORES):
        # x8p[p, j, s, n] = x[c*BL+s, n, 2p+j]
        xs = x_np[c * BL : (c + 1) * BL]                     # [BL, H, E]
        x8p = np.ascontiguousarray(
            xs.transpose(2, 0, 1).reshape(64, 2, BL, H)
        ).astype(NP_FP8)
        # interleaved stripe shard: node(k, i) = 512*k + 64*c + i
        nodes = (512 * np.arange(4)[:, None] + 64 * c
                 + np.arange(64)[None, :]).reshape(-1)
        w1s = np.ascontiguousarray(W1r[nodes].transpose(1, 0, 2)).astype(NP_BF16)
        in_maps.append({
            "x8p": x8p, "adj8": adj8, "wc8": wc8, "cb": cb,
            "gw1": gw1, "gb1": gb1_, "gw2": gw2, "gb2": gb2_,
            "dsc64": dsc64, "dsc256": dsc256, "dinvrow": dinvrow,
            "w1s": w1s, "b1r": b1r, "w2r": w2r, "b2r": b2r,
        })
    return in_maps


_NC_CACHE = {}


def kernel(**inputs) -> np.ndarray:
    key = "full"
    if key not in _NC_CACHE:
        _NC_CACHE[key] = build_nc()
    nc = _NC_CACHE[key]
    in_maps = _prep_inputs(**inputs)
    res = run_bass_kernel_spmd(nc, in_maps, core_ids=list(range(NCORES)))
    out = np.concatenate([res.results[c]["out"] for c in range(NCORES)], axis=0)
    return out.astype(np.float32)
